# revision 11
# baseline (speedup 1.0000x reference)
"""Trainium2 Bass kernel for DiagonalVariational sampling.

z[n, i] = m[i] + std_normal[n, i] * (diag_L[i] + JITTER)

Sharding: std_normal split along n_sample across 8 cores (data parallel);
m and diag_L replicated. Pure elementwise -> memory-bound (~64 MiB HBM
traffic per core, ~410 GB/s aggregate DMA => ~165 us floor).

Layout trick: process 16-sample row blocks as [128, 2048] tiles via the
flat contiguous reshape (partition p of a tile holds d-range
[(p%8)*2048, (p%8+1)*2048) of row p//8). The diag_L/m operands then
repeat with period 8 along partitions, so a single pair of [128, 2048]
SBUF tiles (built once by a replicated HBM DMA) serves every tile --
no PE broadcast, no per-chunk re-broadcast stalls.

Per tile: STT (scale+JITTER)*x then +m, tiles split DVE:GPSIMD ~2:1 so
total compute (~90 us) stays far under the DMA floor. Loads stream on
the sync queue, stores on the scalar queue; both share the 16 DMA
engines continuously.
"""

import numpy as np

import concourse.bacc as bacc
import concourse.mybir as mybir
import concourse.tile as tile
from concourse.bass_utils import run_bass_kernel_spmd

D = 16384
N_SAMPLE = 4096
N_CORES = 8
ROWS = N_SAMPLE // N_CORES  # 512 sample rows per core
P = 128                     # SBUF partitions
G = 16                      # sample rows per tile
NT = ROWS // G              # 32 tiles per core
F = G * D // P              # 2048 free elems per partition per tile
Q = D // F                  # 8: d-chunk period along partitions
JITTER = 1e-06
DT = mybir.dt.float32

_CACHE: dict = {}


def _build_nc(repeats=1, variant="v2", xbufs=10):
    nc = bacc.Bacc(
        "TRN2", target_bir_lowering=False, debug=False, num_devices=N_CORES
    )
    m = nc.dram_tensor("m", [D], DT, kind="ExternalInput")
    dl = nc.dram_tensor("diag_L", [D], DT, kind="ExternalInput")
    x = nc.dram_tensor("x", [ROWS, D], DT, kind="ExternalInput")
    z = nc.dram_tensor("z", [ROWS, D], DT, kind="ExternalOutput")

    with tile.TileContext(nc) as tc:
        with (
            tc.tile_pool(name="const", bufs=1) as cpool,
            tc.tile_pool(name="xt", bufs=xbufs) as xpool,
        ):
            scale_b = cpool.tile([P, F], DT)  # diag_L, period-Q replicated
            m_b = cpool.tile([P, F], DT)      # m, period-Q replicated

            # Tiles use partition order (q a): partition p holds d-chunk
            # q=p//G of row a=p%G, so the diag_L/m operands are constant
            # over contiguous 16-partition blocks. Replicate each chunk
            # with one broadcast DMA per block, on the scalar (store)
            # queue, which is idle at kernel start.
            for q in range(Q):
                sl = slice(q * F, (q + 1) * F)
                ps = slice(q * G, (q + 1) * G)
                nc.scalar.dma_start(
                    out=scale_b[ps, :],
                    in_=dl[sl].rearrange("(a f) -> a f", a=1).broadcast_to(
                        [G, F]
                    ),
                )
                nc.scalar.dma_start(
                    out=m_b[ps, :],
                    in_=m[sl].rearrange("(a f) -> a f", a=1).broadcast_to(
                        [G, F]
                    ),
                )

            # Fold the jitter into scale_b once; per-tile ops are then
            # plain tensor_tensor (Pool has no scalar_tensor_tensor).
            nc.vector.tensor_scalar_add(scale_b[:], scale_b[:], JITTER)

            # Pool (gpsimd) handles 3 of every 8 tiles: DVE is ~2x faster
            # per op, so a 5:3 tile split balances the two engines.
            pool_tiles = {2, 5, 7}
            for _r in range(repeats):
                for i in range(NT):
                    rs = slice(i * G, (i + 1) * G)
                    xt = xpool.tile([P, F], DT)
                    nc.sync.dma_start(
                        out=xt[:],
                        in_=x[rs, :].rearrange("a (q f) -> q a f", q=Q),
                    )
                    eng = (
                        nc.gpsimd if (i % 8) in pool_tiles and variant == "v2"
                        else nc.vector
                    )
                    # xt = scale * xt; xt += m
                    eng.tensor_mul(xt[:], xt[:], scale_b[:])
                    eng.tensor_add(xt[:], xt[:], m_b[:])
                    nc.scalar.dma_start(
                        out=z[rs, :].rearrange("a (q f) -> q a f", q=Q),
                        in_=xt[:],
                    )

    nc.compile()
    return nc


def get_nc(repeats=1, variant="v2", xbufs=10):
    key = (repeats, variant, xbufs)
    if key not in _CACHE:
        _CACHE[key] = _build_nc(repeats, variant, xbufs)
    return _CACHE[key]


def run_spmd(m, diag_L, std_normal, trace=False, repeats=1, variant="v2",
             xbufs=10):
    """Run the SPMD kernel; returns (z_full, BassKernelResults)."""
    nc = get_nc(repeats, variant, xbufs)
    m = np.ascontiguousarray(m, dtype=np.float32)
    diag_L = np.ascontiguousarray(diag_L, dtype=np.float32)
    std_normal = np.ascontiguousarray(std_normal, dtype=np.float32)
    in_maps = [
        {
            "m": m,
            "diag_L": diag_L,
            "x": std_normal[i * ROWS : (i + 1) * ROWS],
        }
        for i in range(N_CORES)
    ]
    res = run_bass_kernel_spmd(nc, in_maps, list(range(N_CORES)), trace=trace)
    z = np.concatenate([res.results[i]["z"] for i in range(N_CORES)], axis=0)
    return z, res


def kernel(m, diag_L, std_normal):
    z, _ = run_spmd(m, diag_L, std_normal)
    return z
